# revision 12
# baseline (speedup 1.0000x reference)
"""Depth-aware 3x3 conv (depth-similarity modulated conv) on 8 Trainium2
NeuronCores, batch-parallel (1 image per core).

out[b,o,h,w] = sum_{c,k} weight[o,c,k] * fd[b,k,h,w] * xpatch[b,c,k,h,w] + bias
fd[k,p] = exp(-8.3 * |depth[p + delta_k] - depth[p]|)   (zero-padded patches)

v2 design (per core, image [64, 256, 256] fp16):
- Padded plane flattened: q = (h+1)*258 + (w+1), NP = 258*258.
- 9 taps; 8 modulated taps in 4 partition-pair tiles (2 taps x 64ch = K=128
  matmuls), center tap unmodulated K=64. Pair tiles:
    T1 = [x ; x@+1]   pairs A=(t0,t1) m0=1,  B=(t7,t8) m0=518
    T2 = [x ; x@+256] pairs C=(t2,t3) m0=3,  D=(t5,t6) m0=261
  x loaded from HBM ONCE per chunk ([64, 6152]); T1-upper / T2 halves are
  built with DVE tensor_copy (fp16 4x) cross-quadrant writes.
- fd computed packed [88 = 8 taps x 11 segs, 512] fp32->fp16 (sub DVE, abs+exp
  ACT).
- fd fanout to 64 channel rows:
    pairs A,B: DRAM round-trip (4 lines) + one [128, CH] stride-0 DMA per pair
    pairs C,D: PE broadcast matmul (lhsT = E2 [2,128] ones-blocks, rhs = fdp
      rows [2, 512]) -> PSUM, ACT copies PSUM->SBUF fp16. No DMA.
- Modulate (x * fd_rep): pairs A,B on GPSIMD ([128, CH] tensor_tensor fp16),
  pairs C,D on DVE per 512-group.
- Mains per 512-group: 5 accumulating matmuls (4 pairs K=128 + center K=64),
  PE bcasts for group g+1 interleaved; ScalarE evicts with bias -> fp16.
"""
import numpy as np

import concourse.bacc as bacc
import concourse.bass as bass
import concourse.mybir as mybir
import concourse.tile as tile
from concourse.bass_utils import run_bass_kernel_spmd

F16 = mybir.dt.float16
F32 = mybir.dt.float32

B, C, H, W = 8, 64, 256, 256
Hp, Wp = H + 2, W + 2          # 258
NP = Hp * Wp                   # 66564
ALPHA = 8.3

GW = 512                       # matmul group width (psum bank)
SEGS = 11                      # fd segments per chunk
CH = GW * SEGS                 # 5632 output pixels per chunk
NCHUNK = 12                    # 12*5632 = 67584 >= NP
OUTW = NCHUNK * CH

XSL, XSH = 512, 4608           # x line slacks (elements), as baseline
DSL, DSH = 512, 4608
XW = XSL + NP + XSH
DW = DSL + NP + DSH

XT_W = CH + 520                # 6152: x stream width (max col 518+CH, +1 shift)
T2_W = CH + 262                # 5894: T2 tile width (max col 261+CH+1)

# tap id t = kh*3+kw, delta = (kh-1)*258 + (kw-1)
DELTA = [(kh - 1) * Wp + (kw - 1) for kh in range(3) for kw in range(3)]
# pairs (ta, tb, tile_idx, m0): tb's shift baked into the tile's upper half
PAIRS = [(0, 1, 0, 1), (7, 8, 0, 518), (2, 3, 1, 3), (5, 6, 1, 261)]
# modulated-tap order in fdp packing: [t0,t1,t2,t3,t5,t6,t7,t8]
TAP8 = [0, 1, 2, 3, 5, 6, 7, 8]
# fd DRAM lines (pairs A, B): line l <- tap
FD_LINES = [0, 1, 7, 8]        # fdp blocks 0:11, 11:22, 66:77, 77:88
LW = OUTW                      # fd line width


def _build_nc():
    nc = bacc.Bacc("TRN2", target_bir_lowering=False, debug=False, num_devices=8)
    x_line = nc.declare_dram_parameter("x_line", [C, XW], F16, isOutput=False)
    d_line = nc.declare_dram_parameter("d_line", [1, DW], F32, isOutput=False)
    wts = nc.declare_dram_parameter("wts", [128, 448], F16, isOutput=False)
    bias = nc.declare_dram_parameter("bias", [64, 1], F32, isOutput=False)
    out_l = nc.declare_dram_parameter("out_line", [C, OUTW], F16, isOutput=True)

    x_t = x_line.ap().tensor
    d_t = d_line.ap().tensor
    fd_dram = nc.dram_tensor("fd_scratch", [4, LW], F16)
    fd_t = fd_dram.ap().tensor

    with tile.TileContext(nc) as tc:
        with (
            tc.tile_pool(name="const", bufs=1) as cpool,
            tc.tile_pool(name="xt", bufs=2) as xpool,
            tc.tile_pool(name="fdgen", bufs=2) as gpool,
            tc.tile_pool(name="fr", bufs=2) as fpool,
            tc.tile_pool(name="frsb", bufs=3) as fspool,
            tc.tile_pool(name="mmod", bufs=2) as mpool,
            tc.tile_pool(name="mcd", bufs=3) as mcdpool,
            tc.tile_pool(name="ost", bufs=2) as opool,
            tc.tile_pool(name="ps", bufs=2, space="PSUM") as pspool,
            tc.tile_pool(name="psfr", bufs=4, space="PSUM") as pfpool,
        ):
            wt_sb = cpool.tile([128, 448], F16, tag="w")
            nc.sync.dma_start(wt_sb[:], wts[:])
            bias_sb = cpool.tile([64, 1], F32, tag="b")
            nc.sync.dma_start(bias_sb[:], bias[:])

            for i in range(NCHUNK):
                q0 = i * CH
                xbase = XSL + q0 - 260

                # ---- x: one HBM stream, shifted copies on DVE ----
                t1 = xpool.tile([128, XT_W], F16, tag="t1")
                nc.sync.dma_start(
                    t1[0:64, :],
                    bass.AP(x_t, xbase, [[XW, 64], [1, XT_W]]))
                t2 = xpool.tile([128, T2_W], F16, tag="t2")
                nc.vector.tensor_copy(t1[64:128, 0:XT_W - 1],
                                      t1[0:64, 1:XT_W])
                nc.vector.tensor_copy(t2[0:64, :], t1[0:64, 0:T2_W])
                nc.vector.tensor_copy(t2[64:128, :],
                                      t1[0:64, 256:256 + T2_W])

                # ---- fd generation, packed [88, 512]:
                #   A/B taps tap-major: t0 0:11, t1 11:22, t7 22:33, t8 33:44
                #   C/D taps seg-major: 44+4g+{0:t2, 1:t3, 2:t5, 3:t6}
                dp = gpool.tile([88, GW], F32, tag="dp")
                nc.sync.dma_start(
                    dp[0:22, :],
                    bass.AP(d_t, DSL + q0 - 259,
                            [[1, 2], [GW, SEGS], [1, GW]]))
                nc.sync.dma_start(
                    dp[22:44, :],
                    bass.AP(d_t, DSL + q0 + 258,
                            [[1, 2], [GW, SEGS], [1, GW]]))
                for j, dlt in enumerate((-257, -1, 1, 257)):
                    nc.sync.dma_start(
                        dp[44 + j:88:4, :],
                        bass.AP(d_t, DSL + q0 + dlt, [[GW, SEGS], [1, GW]]))
                dc = gpool.tile([88, GW], F32, tag="dc")
                nc.sync.dma_start(
                    dc[0:44, :],
                    bass.AP(d_t, DSL + q0,
                            [[0, 4], [GW, SEGS], [1, GW]]))
                nc.sync.dma_start(
                    dc[44:88, :],
                    bass.AP(d_t, DSL + q0,
                            [[GW, SEGS], [0, 4], [1, GW]]))
                df = gpool.tile([88, GW], F32, tag="df")
                nc.vector.tensor_tensor(df[:], dp[:], dc[:],
                                        mybir.AluOpType.subtract)
                da = gpool.tile([88, GW], F32, tag="da")
                nc.scalar.activation(da[:], df[:],
                                     mybir.ActivationFunctionType.Abs)
                fdp = gpool.tile([88, GW], F16, tag="fdp")
                nc.scalar.activation(fdp[:], da[:],
                                     mybir.ActivationFunctionType.Exp,
                                     scale=-ALPHA)

                # ---- pairs A,B: fd lines to DRAM, stride-0 fanout DMA ----
                for l in range(4):
                    blk = l * SEGS
                    nc.sync.dma_start(
                        bass.AP(fd_t, l * LW + q0, [[GW, SEGS], [1, GW]]),
                        fdp[blk:blk + SEGS, :])
                # ---- pairs C,D: reshape fd rows to [2, CH] tiles (base 0) ----
                fdc = gpool.tile([2, CH], F16, tag="fdc", bufs=1)
                fdd = gpool.tile([2, CH], F16, tag="fdd", bufs=1)
                for j, dst in enumerate((fdc[0:1, :], fdc[1:2, :],
                                         fdd[0:1, :], fdd[1:2, :])):
                    nc.sync.dma_start(dst, fdp[44 + j:88:4, :])
                frs = []
                for pi in range(2):          # A then B
                    fr = fpool.tile([128, CH], F16, tag=f"fr{pi}")
                    nc.sync.dma_start(
                        fr[:],
                        bass.AP(fd_t, 2 * pi * LW + q0,
                                [[LW, 2], [0, 64], [1, CH]]))
                    frs.append(fr)

                # ---- pairs A,B modulate on GPSIMD ----
                mtA = mpool.tile([128, CH], F16, tag="mtA")
                nc.gpsimd.tensor_tensor(mtA[:], t1[:, 1:1 + CH], frs[0][:],
                                        mybir.AluOpType.mult)
                mtB = mpool.tile([128, CH], F16, tag="mtB")
                nc.gpsimd.tensor_tensor(mtB[:], t1[:, 518:518 + CH], frs[1][:],
                                        mybir.AluOpType.mult)

                # ---- groups: bcast C/D (PE) + copies + modulate + mains ----
                ost = opool.tile([64, CH], F16, tag="o")
                mcds = {}
                for g in range(SEGS + 1):
                    if g < SEGS:
                        # broadcast fd rows for pairs C=(t2,t3), D=(t5,t6)
                        frcd = []
                        for src2 in (fdc, fdd):
                            fps = pfpool.tile([128, GW], F32)
                            nc.tensor.matmul(
                                fps[:], wt_sb[0:2, 320:448],
                                src2[:, g * GW:(g + 1) * GW],
                                start=True, stop=True,
                                tile_position=(0, 0))
                            fsb = fspool.tile([128, GW], F16, tag="fsb")
                            nc.scalar.activation(
                                fsb[:], fps[:],
                                mybir.ActivationFunctionType.Identity)
                            frcd.append(fsb)
                        # modulate C/D for group g on DVE
                        mc = mcdpool.tile([128, GW], F16, tag="mc")
                        nc.vector.tensor_tensor(
                            mc[:], t2[:, 3 + g * GW: 3 + (g + 1) * GW],
                            frcd[0][:], mybir.AluOpType.mult)
                        md = mcdpool.tile([128, GW], F16, tag="md")
                        nc.vector.tensor_tensor(
                            md[:], t2[:, 261 + g * GW: 261 + (g + 1) * GW],
                            frcd[1][:], mybir.AluOpType.mult)
                        mcds[g] = (mc, md)

                    if g > 0:
                        # mains for group g-1
                        h = g - 1
                        ps = pspool.tile([64, GW], F32)
                        nc.tensor.matmul(
                            ps[:], wt_sb[:, 0:64],
                            mtA[:, h * GW:(h + 1) * GW],
                            start=True, stop=False)
                        nc.tensor.matmul(
                            ps[:], wt_sb[:, 64:128],
                            mtB[:, h * GW:(h + 1) * GW],
                            start=False, stop=False)
                        mc, md = mcds.pop(h)
                        nc.tensor.matmul(
                            ps[:], wt_sb[:, 128:192], mc[:],
                            start=False, stop=False)
                        nc.tensor.matmul(
                            ps[:], wt_sb[:, 192:256], md[:],
                            start=False, stop=False)
                        nc.tensor.matmul(
                            ps[:], wt_sb[0:64, 256:320],
                            t1[0:64, 260 + h * GW: 260 + (h + 1) * GW],
                            start=False, stop=True)
                        nc.scalar.activation(
                            ost[:, h * GW:(h + 1) * GW], ps[:],
                            mybir.ActivationFunctionType.Identity,
                            bias=bias_sb[:], scale=1.0)
                nc.sync.dma_start(out_l[:, q0:q0 + CH], ost[:])
    nc.compile()
    return nc


_NC_CACHE = None


def _get_nc():
    global _NC_CACHE
    if _NC_CACHE is None:
        _NC_CACHE = _build_nc()
    return _NC_CACHE


def _make_in_maps(inputs):
    x = np.asarray(inputs["x"], dtype=np.float32)
    depth = np.asarray(inputs["depth"], dtype=np.float32)
    weight = np.asarray(inputs["weight"], dtype=np.float32)
    bias_np = np.asarray(inputs["bias"], dtype=np.float32)

    xl = np.zeros((B, C, XW), np.float16)
    xpad = np.zeros((B, C, Hp, Wp), np.float32)
    xpad[:, :, 1:257, 1:257] = x
    xl[:, :, XSL:XSL + NP] = xpad.reshape(B, C, NP).astype(np.float16)

    dl = np.zeros((B, 1, DW), np.float32)
    dpad = np.zeros((B, Hp, Wp), np.float32)
    dpad[:, 1:257, 1:257] = depth[:, 0]
    dl[:, 0, DSL:DSL + NP] = dpad.reshape(B, NP)

    wts = np.zeros((128, 448), np.float16)
    for g, (ta, tb, _, _) in enumerate(PAIRS):
        # lhsT[c, o] = weight[o, c, kh, kw]
        wts[0:64, g * 64:(g + 1) * 64] = \
            weight[:, :, ta // 3, ta % 3].T.astype(np.float16)
        wts[64:128, g * 64:(g + 1) * 64] = \
            weight[:, :, tb // 3, tb % 3].T.astype(np.float16)
    wts[0:64, 256:320] = weight[:, :, 1, 1].T.astype(np.float16)
    # E2 broadcast matrix: psum rows 0:64 <- rhs row 0, rows 64:128 <- row 1
    wts[0, 320:384] = 1.0
    wts[1, 384:448] = 1.0

    bias_col = bias_np.reshape(64, 1)
    return [
        {"x_line": xl[b], "d_line": dl[b], "wts": wts, "bias": bias_col}
        for b in range(B)
    ]


def kernel(x, depth, weight, bias):
    nc = _get_nc()
    in_maps = _make_in_maps(
        {"x": x, "depth": depth, "weight": weight, "bias": bias})
    res = run_bass_kernel_spmd(nc, in_maps, list(range(B)))

    out = np.empty((B, C, H, W), np.float32)
    for b in range(B):
        ol = res.results[b]["out_line"][:, :NP].astype(np.float32)
        out[b] = ol.reshape(C, Hp, Wp)[:, 1:257, 1:257]
    return out
